# revision 25
# baseline (speedup 1.0000x reference)
"""Trainium2 Bass kernel for nn_JResCOPAttn (B=1, L=1024, D=128).

Reference computation:
    a   = x @ Wl.T + bl                                # [L, D]
    tm  = (a[:,None,:] * a[None,:,:]) @ Wlo.T + blo    # [L, L, D]  (never materialized!)
    tm *= (mask != 0)
    tx  = x @ Wl2.T + bl2                              # [L, D]
    y   = x + einsum('cad,ad->cd', tm, tx)
    out = LayerNorm(y) * gamma + beta

Algebraic restructuring (e indexes the D channels of `a`):
    y1[c,d] = sum_e a[c,e] * T'[c,e,d]  +  blo[d] * Z[c,d]
    T'[c,e,d] = sum_a mask[c,a] * (a[a,e] * tx[a,d] * Wlo[d,e])
    Z[c,d]    = sum_a mask[c,a] * tx[a,d]

Sharding: the e axis (128 channels) is split across the 8 cores (16 each).
Every core computes its 16-channel partial y1 for ALL 1024 output rows:
    U'[a, j, d] = a[a, e0+j] * tx[a,d] * Wlo[d, e0+j]   (bf16, DVE fused op)
    T' = mask @ U'                                      (bf16 matmuls, N=512)
    partial[c,d] = sum_j a[c, e0+j] * T'[c,j,d]         (ACT scale + DVE tree)
then a ReduceScatter over HBM sums the 8 partials and hands each core its own
128-row c-shard, where the Z term, residual and LayerNorm are applied.
The per-core e-chunk enters only through input tensors (WlTc / WloBc), so a
single compiled program serves all 8 cores.
"""

import os
import sys

for _p in ("/opt/trn_rl_repo", "/root/.axon_site/_ro/trn_rl_repo"):
    if os.path.isdir(_p) and _p not in sys.path:
        sys.path.insert(0, _p)

import numpy as np
import ml_dtypes

import concourse.tile as tile
from concourse import bacc, mybir
from concourse.bass_utils import run_bass_kernel_spmd

B, L, D = 1, 1024, 128
NCORES = 8
CB = L // NCORES          # c-rows per core shard = 128
T = L // 128              # a-tiles / c-tiles = 8
EC = D // NCORES          # e-channels per core = 16
EPS = 1e-5
FP = mybir.dt.float32
BF = mybir.dt.bfloat16

NSPLIT = 4                # 512-wide n-chunks per (ct, t) matmul group


def build_nc():
    nc = bacc.Bacc("TRN2", target_bir_lowering=False, num_devices=NCORES,
                   num_swdge_queues=4)

    # ---- I/O (per-core tensors; e-chunk/c-shard baked into the data) ----
    xT    = nc.dram_tensor("xT",    [128, L], FP, kind="ExternalInput")      # x^T
    xrow  = nc.dram_tensor("xrow",  [CB, D], FP, kind="ExternalInput")       # own c-shard of x
    mT    = nc.dram_tensor("mT",    [128, T, L], BF, kind="ExternalInput")   # mT[p,t,c] = mask[c, t*128+p]
    mTc   = nc.dram_tensor("mTc",   [128, T, CB], BF, kind="ExternalInput")  # own-shard mask cols
    WcatT = nc.dram_tensor("WcatT", [128, 128 + EC], FP, kind="ExternalInput")  # [Wl2.T | Wl.T[:,e0:e0+EC]]
    WloBc = nc.dram_tensor("WloBc", [128, EC * 128], BF, kind="ExternalInput")  # Wlo[d, e0+j], bcast parts
    blcB  = nc.dram_tensor("blcB",  [128, EC], FP, kind="ExternalInput")     # bl[e0:e0+EC] bcast parts
    bl2B  = nc.dram_tensor("bl2B",  [128, 128], FP, kind="ExternalInput")    # bl2 bcast parts
    bloB  = nc.dram_tensor("bloB",  [CB, D], FP, kind="ExternalInput")       # blo bcast parts
    gam   = nc.dram_tensor("gam",   [CB, D], FP, kind="ExternalInput")
    bet   = nc.dram_tensor("bet",   [CB, D], FP, kind="ExternalInput")
    out   = nc.dram_tensor("out",   [CB, D], FP, kind="ExternalOutput")

    Sqrt = mybir.ActivationFunctionType.Sqrt
    mult = mybir.AluOpType.mult
    add = mybir.AluOpType.add
    bypass = mybir.AluOpType.bypass

    with tile.TileContext(nc) as tc:
        with (
            tc.tile_pool(name="singles", bufs=1) as singles,
            tc.tile_pool(name="mm", bufs=2, space="PSUM") as mmps,
            tc.tile_pool(name="v", bufs=2) as vpool,
            tc.tile_pool(name="tr", bufs=2) as trpool,
            tc.tile_pool(name="dram", bufs=1, space="DRAM") as dram,
        ):
            # ---- load inputs ----
            sb_xT = singles.tile([128, L], FP)
            nc.sync.dma_start(sb_xT, xT[:, :])
            sb_WcatT = singles.tile([128, 128 + EC], FP)
            nc.sync.dma_start(sb_WcatT, WcatT[:, :])
            sb_WloBc = singles.tile([128, EC * 128], BF)
            nc.sync.dma_start(sb_WloBc, WloBc[:, :])
            sb_blcB = singles.tile([128, EC], FP)
            nc.sync.dma_start(sb_blcB, blcB[:, :])
            sb_bl2B = singles.tile([128, 128], FP)
            nc.sync.dma_start(sb_bl2B, bl2B[:, :])
            sb_bloB = singles.tile([CB, D], FP)
            nc.sync.dma_start(sb_bloB, bloB[:, :])
            sb_xrow = singles.tile([CB, D], FP)
            nc.sync.dma_start(sb_xrow, xrow[:, :])
            sb_gam = singles.tile([CB, D], FP)
            nc.sync.dma_start(sb_gam, gam[:, :])
            sb_bet = singles.tile([CB, D], FP)
            nc.sync.dma_start(sb_bet, bet[:, :])
            sb_eps = singles.tile([CB, 1], FP)
            nc.vector.memset(sb_eps, EPS)
            sb_mT = singles.tile([128, T, L], BF)
            nc.sync.dma_start(sb_mT, mT[:, :, :])
            sb_mTc = singles.tile([128, T, CB], BF)
            nc.sync.dma_start(sb_mTc, mTc[:, :, :])

            # ---- tiny warmup AllToAll: opens the collective channels while
            # compute runs so the real exchange pays less entry latency ----
            warm_in = dram.tile([NCORES * 8, D], BF)
            warm_out = dram.tile([NCORES * 8, D], BF)
            nc.gpsimd.collective_compute(
                "AllToAll", bypass,
                replica_groups=[list(range(NCORES))],
                ins=[warm_in[:, :].opt()],
                outs=[warm_out[:, :].opt()],
            )

            # ---- activations in natural layout (rows on partitions), bf16 ----
            # tx_nat[p, t, d] = tx[t*128+p, d];  a_sel[p, t, j] = a[t*128+p, e0+j]
            tx_nat = singles.tile([128, T, 128], BF)
            a_sel = singles.tile([128, T, EC], FP)    # fp32: ACT scale requirement
            for h in range(2):
                ps = mmps.tile([128, NSPLIT, 512], FP, tag="mm")
                for q in range(4):
                    t = h * 4 + q
                    sl = slice(t * 128, (t + 1) * 128)
                    nc.tensor.matmul(ps[:, q, 0:128 + EC], sb_xT[:, sl], sb_WcatT,
                                     start=True, stop=True)
                for q in range(4):
                    t = h * 4 + q
                    nc.vector.tensor_add(tx_nat[:, t, :], ps[:, q, 0:128], sb_bl2B)
                    nc.vector.tensor_add(a_sel[:, t, :], ps[:, q, 128:128 + EC],
                                         sb_blcB)

            # ---- Z term for own shard: Z[c,d] = sum_a mask[c,a] tx[a,d] ----
            z_ps = mmps.tile([128, NSPLIT, 512], FP, tag="mm")
            for t in range(T):
                nc.tensor.matmul(z_ps[:, 0, 0:CB], sb_mTc[:, t, :], tx_nat[:, t, :],
                                 start=(t == 0), stop=(t == T - 1))
            # base = x_shard + blo * Z   (everything not needing the collective)
            base = singles.tile([CB, D], FP)
            nc.vector.scalar_tensor_tensor(base, z_ps[:, 0, 0:CB], 1.0, sb_bloB,
                                           op0=mult, op1=mult)
            nc.vector.tensor_add(base, base, sb_xrow)

            # ---- U'[a, j, d] = a[a,e0+j] * tx[a,d] * Wlo[d,e0+j]  (bf16) ----
            # two stages: atx = a*tx (tensor_scalar, 4x mode), then one big
            # tensor_tensor per a-tile against the flat Wlo broadcast (2x mode)
            atx = singles.tile([128, T, EC * 128], BF)
            up = singles.tile([128, T, EC * 128], BF)
            N_DVE_SCALE = 11   # of the 16 per-tile a-scales, how many go to DVE
            for t in range(T):
                for j in range(EC):
                    dst = atx[:, t, j * 128:(j + 1) * 128]
                    sca = a_sel[:, t, j:j + 1]
                    if j < N_DVE_SCALE:
                        nc.vector.tensor_scalar_mul(dst, tx_nat[:, t, :], sca)
                    else:
                        nc.scalar.mul(dst, tx_nat[:, t, :], sca)
                nc.vector.tensor_mul(up[:, t, :], atx[:, t, :], sb_WloBc)

            # ---- main loop over output c-tiles. The first two c-tiles are
            # interleaved with the a-tile loop outermost so the matmuls
            # consume U' tiles as the vector engines produce them; the rest
            # run sequentially with PSUM slot rotation hiding the V drain ----
            part_dram = dram.tile([L, D], BF)
            a2a_dram = dram.tile([L, D], BF)

            def v_stage(ct, ps, n_act=6):
                # V[c, j, d] = a[c, e0+j] * T'[c, j, d]  (ACT/DVE, PSUM->SBUF)
                v = vpool.tile([128, EC, 128], BF, tag="v")
                for j in range(EC):
                    src_ap = ps[:, j // 4, (j % 4) * 128:(j % 4 + 1) * 128]
                    sca = a_sel[:, ct, j:j + 1]
                    if j < n_act:
                        nc.scalar.mul(v[:, j, :], src_ap, sca)
                    else:
                        nc.vector.tensor_scalar_mul(v[:, j, :], src_ap, sca)
                # tree-reduce over j -> partial[c, d]
                t8 = trpool.tile([128, 8, 128], BF, tag="t8")
                nc.vector.tensor_add(t8, v[:, 0:8, :], v[:, 8:16, :])
                t4 = trpool.tile([128, 4, 128], BF, tag="t4")
                nc.vector.tensor_add(t4, t8[:, 0:4, :], t8[:, 4:8, :])
                t2 = trpool.tile([128, 2, 128], BF, tag="t2")
                nc.vector.tensor_add(t2, t4[:, 0:2, :], t4[:, 2:4, :])
                p1 = trpool.tile([128, 128], BF, tag="p1")
                nc.vector.tensor_add(p1, t2[:, 0, :], t2[:, 1, :])
                nc.sync.dma_start(part_dram[ct * 128:(ct + 1) * 128, :], p1)

            ps0 = mmps.tile([128, NSPLIT, 512], FP, tag="mm")
            ps1 = mmps.tile([128, NSPLIT, 512], FP, tag="mm")
            for t in range(T):
                for ct, ps in ((0, ps0), (1, ps1)):
                    lhsT = sb_mT[:, t, ct * 128:(ct + 1) * 128]
                    for n in range(NSPLIT):
                        nc.tensor.matmul(
                            ps[:, n, :], lhsT, up[:, t, n * 512:(n + 1) * 512],
                            start=(t == 0), stop=(t == T - 1),
                        )
            v_stage(0, ps0)
            v_stage(1, ps1)

            for ct in range(2, T):
                ps = mmps.tile([128, NSPLIT, 512], FP, tag="mm")
                for t in range(T):
                    lhsT = sb_mT[:, t, ct * 128:(ct + 1) * 128]
                    for n in range(NSPLIT):
                        nc.tensor.matmul(
                            ps[:, n, :], lhsT, up[:, t, n * 512:(n + 1) * 512],
                            start=(t == 0), stop=(t == T - 1),
                        )
                v_stage(ct, ps)

            nc.gpsimd.collective_compute(
                "AllToAll", bypass,
                replica_groups=[list(range(NCORES))],
                ins=[part_dram[:, :].opt()],
                outs=[a2a_dram[:, :].opt()],
            )
            # AllToAll leaves, on core k, the 8 cores' partials of this
            # core's 128-row c-shard: [i, 128, D] with source core i.
            red_sb = singles.tile([128, NCORES, D], BF)
            nc.sync.dma_start(
                red_sb, a2a_dram[:, :].rearrange("(s p) d -> p s d", p=128))
            r4 = singles.tile([128, 4, D], BF)
            nc.vector.tensor_add(r4, red_sb[:, 0:4, :], red_sb[:, 4:8, :])
            r2 = singles.tile([128, 2, D], BF)
            nc.vector.tensor_add(r2, r4[:, 0:2, :], r4[:, 2:4, :])
            y_sb = singles.tile([CB, D], FP)
            nc.vector.tensor_add(y_sb, r2[:, 0, :], r2[:, 1, :])

            # ---- y = base + reduced partials ; LayerNorm ----
            nc.vector.tensor_add(y_sb, y_sb, base)

            stats = singles.tile([CB, nc.vector.BN_STATS_DIM], FP)
            nc.vector.bn_stats(stats, y_sb)
            mv = singles.tile([CB, 2], FP)
            nc.vector.bn_aggr(mv, stats)
            sd = singles.tile([CB, 1], FP)
            nc.scalar.activation(sd, mv[:, 1:2], Sqrt, bias=sb_eps, scale=1.0)
            rstd = singles.tile([CB, 1], FP)
            nc.vector.reciprocal(rstd, sd)
            nc.vector.tensor_scalar(y_sb, y_sb, mv[:, 0:1], rstd,
                                    op0=mybir.AluOpType.subtract, op1=mult)
            nc.vector.tensor_mul(y_sb, y_sb, sb_gam)
            nc.vector.tensor_add(y_sb, y_sb, sb_bet)

            nc.sync.dma_start(out[:, :], y_sb)

    return nc


_NC_CACHE = None


def _get_nc():
    global _NC_CACHE
    if _NC_CACHE is None:
        _NC_CACHE = build_nc()
        _NC_CACHE.finalize()
    return _NC_CACHE


def _prepare_in_maps(x, mask, Wl, bl, Wlo, blo, Wl2, bl2, gamma, beta):
    f32 = np.float32
    bf16 = ml_dtypes.bfloat16
    x0 = np.ascontiguousarray(np.asarray(x, f32)[0])          # [L, D]
    xT = np.ascontiguousarray(x0.T)                           # [128, L]
    m = np.asarray(mask)[0].astype(bf16)                      # [L(c), L(a)]
    # mT[p, t, c] = mask[c, t*128 + p]
    mT_full = np.ascontiguousarray(
        m.T.reshape(T, 128, L).transpose(1, 0, 2))            # [128, T, L]
    WlT = np.ascontiguousarray(np.asarray(Wl, f32).T)         # [in, e]
    Wl2T = np.ascontiguousarray(np.asarray(Wl2, f32).T)
    WloT = np.asarray(Wlo, f32).T                             # [e, d]
    bl_ = np.asarray(bl, f32)
    bl2B = np.ascontiguousarray(np.broadcast_to(np.asarray(bl2, f32), (128, 128)))
    bloB = np.ascontiguousarray(np.broadcast_to(np.asarray(blo, f32), (CB, D)))
    gam_b = np.ascontiguousarray(np.broadcast_to(np.asarray(gamma, f32), (CB, D)))
    bet_b = np.ascontiguousarray(np.broadcast_to(np.asarray(beta, f32), (CB, D)))

    in_maps = []
    for k in range(NCORES):
        own = np.r_[k * CB:(k + 1) * CB]
        e0 = k * EC
        WloBc = np.ascontiguousarray(
            np.broadcast_to(WloT[e0:e0 + EC].astype(bf16).reshape(1, EC * 128),
                            (128, EC * 128)))
        in_maps.append({
            "xT": xT,
            "xrow": np.ascontiguousarray(x0[own]),
            "mT": mT_full,
            "mTc": np.ascontiguousarray(mT_full[:, :, own]),
            "WcatT": np.ascontiguousarray(
                np.concatenate([Wl2T, WlT[:, e0:e0 + EC]], axis=1)),
            "WloBc": WloBc,
            "blcB": np.ascontiguousarray(
                np.broadcast_to(bl_[e0:e0 + EC], (128, EC))),
            "bl2B": bl2B,
            "bloB": bloB,
            "gam": gam_b,
            "bet": bet_b,
        })
    return in_maps


def kernel(x, mask, Wl, bl, Wlo, blo, Wl2, bl2, gamma, beta):
    in_maps = _prepare_in_maps(x, mask, Wl, bl, Wlo, blo, Wl2, bl2, gamma, beta)
    res = run_bass_kernel_spmd(_get_nc(), in_maps, core_ids=list(range(NCORES)))
    y = np.concatenate([res.results[k]["out"] for k in range(NCORES)], axis=0)
    return y.reshape(B, L, D).astype(np.float32)


# revision 26
# speedup vs baseline: 1.1744x; 1.1744x over previous
"""Trainium2 Bass kernel for nn_JResCOPAttn (B=1, L=1024, D=128).

Reference computation:
    a   = x @ Wl.T + bl                                # [L, D]
    tm  = (a[:,None,:] * a[None,:,:]) @ Wlo.T + blo    # [L, L, D]  (never materialized!)
    tm *= (mask != 0)
    tx  = x @ Wl2.T + bl2                              # [L, D]
    y   = x + einsum('cad,ad->cd', tm, tx)
    out = LayerNorm(y) * gamma + beta

Algebraic restructuring (e indexes the D channels of `a`):
    y1[c,d] = sum_e a[c,e] * T'[c,e,d]  +  blo[d] * Z[c,d]
    T'[c,e,d] = sum_a mask[c,a] * (a[a,e] * tx[a,d] * Wlo[d,e])
    Z[c,d]    = sum_a mask[c,a] * tx[a,d]

Sharding: the e axis (128 channels) is split across the 8 cores (16 each).
Every core computes its 16-channel partial y1 for ALL 1024 output rows:
    U'[a, j, d] = a[a, e0+j] * tx[a,d] * Wlo[d, e0+j]   (bf16, DVE fused op)
    T' = mask @ U'                                      (bf16 matmuls, N=512)
    partial[c,d] = sum_j a[c, e0+j] * T'[c,j,d]         (ACT scale + DVE tree)
then an AllToAll over HBM hands each core the 8 per-core partials of its own
128-row c-shard, which it sums locally before the Z term, residual and
LayerNorm are applied.
The per-core e-chunk enters only through input tensors (WlTc / WloBc), so a
single compiled program serves all 8 cores.
"""

import os
import sys

for _p in ("/opt/trn_rl_repo", "/root/.axon_site/_ro/trn_rl_repo"):
    if os.path.isdir(_p) and _p not in sys.path:
        sys.path.insert(0, _p)

import numpy as np
import ml_dtypes

import concourse.tile as tile
from concourse import bacc, mybir
from concourse.bass_utils import run_bass_kernel_spmd

B, L, D = 1, 1024, 128
NCORES = 8
CB = L // NCORES          # c-rows per core shard = 128
T = L // 128              # a-tiles / c-tiles = 8
EC = D // NCORES          # e-channels per core = 16
EPS = 1e-5
FP = mybir.dt.float32
BF = mybir.dt.bfloat16

NSPLIT = 4                # 512-wide n-chunks per (ct, t) matmul group


def build_nc():
    nc = bacc.Bacc("TRN2", target_bir_lowering=False, num_devices=NCORES,
                   num_swdge_queues=4)

    # ---- I/O (per-core tensors; e-chunk/c-shard baked into the data) ----
    xT    = nc.dram_tensor("xT",    [128, L], FP, kind="ExternalInput")      # x^T
    xrow  = nc.dram_tensor("xrow",  [CB, D], FP, kind="ExternalInput")       # own c-shard of x
    mT    = nc.dram_tensor("mT",    [128, T, L], BF, kind="ExternalInput")   # mT[p,t,c] = mask[c, t*128+p]
    mTc   = nc.dram_tensor("mTc",   [128, T, CB], BF, kind="ExternalInput")  # own-shard mask cols
    WcatT = nc.dram_tensor("WcatT", [128, 128 + EC], FP, kind="ExternalInput")  # [Wl2.T | Wl.T[:,e0:e0+EC]]
    WloBc = nc.dram_tensor("WloBc", [128, EC * 128], BF, kind="ExternalInput")  # Wlo[d, e0+j], bcast parts
    blcB  = nc.dram_tensor("blcB",  [128, EC], FP, kind="ExternalInput")     # bl[e0:e0+EC] bcast parts
    bl2B  = nc.dram_tensor("bl2B",  [128, 128], FP, kind="ExternalInput")    # bl2 bcast parts
    bloB  = nc.dram_tensor("bloB",  [CB, D], FP, kind="ExternalInput")       # blo bcast parts
    gam   = nc.dram_tensor("gam",   [CB, D], FP, kind="ExternalInput")
    bet   = nc.dram_tensor("bet",   [CB, D], FP, kind="ExternalInput")
    out   = nc.dram_tensor("out",   [CB, D], FP, kind="ExternalOutput")

    Sqrt = mybir.ActivationFunctionType.Sqrt
    mult = mybir.AluOpType.mult
    add = mybir.AluOpType.add
    bypass = mybir.AluOpType.bypass

    with tile.TileContext(nc) as tc:
        with (
            tc.tile_pool(name="singles", bufs=1) as singles,
            tc.tile_pool(name="mm", bufs=2, space="PSUM") as mmps,
            tc.tile_pool(name="v", bufs=2) as vpool,
            tc.tile_pool(name="tr", bufs=2) as trpool,
            tc.tile_pool(name="dram", bufs=1, space="DRAM") as dram,
        ):
            # ---- load inputs ----
            sb_xT = singles.tile([128, L], FP)
            nc.sync.dma_start(sb_xT, xT[:, :])
            sb_WcatT = singles.tile([128, 128 + EC], FP)
            nc.sync.dma_start(sb_WcatT, WcatT[:, :])
            sb_WloBc = singles.tile([128, EC * 128], BF)
            nc.sync.dma_start(sb_WloBc, WloBc[:, :])
            sb_blcB = singles.tile([128, EC], FP)
            nc.sync.dma_start(sb_blcB, blcB[:, :])
            sb_bl2B = singles.tile([128, 128], FP)
            nc.sync.dma_start(sb_bl2B, bl2B[:, :])
            sb_bloB = singles.tile([CB, D], FP)
            nc.sync.dma_start(sb_bloB, bloB[:, :])
            sb_xrow = singles.tile([CB, D], FP)
            nc.sync.dma_start(sb_xrow, xrow[:, :])
            sb_gam = singles.tile([CB, D], FP)
            nc.sync.dma_start(sb_gam, gam[:, :])
            sb_bet = singles.tile([CB, D], FP)
            nc.sync.dma_start(sb_bet, bet[:, :])
            sb_eps = singles.tile([CB, 1], FP)
            nc.vector.memset(sb_eps, EPS)
            sb_mT = singles.tile([128, T, L], BF)
            nc.sync.dma_start(sb_mT, mT[:, :, :])
            sb_mTc = singles.tile([128, T, CB], BF)
            nc.sync.dma_start(sb_mTc, mTc[:, :, :])

            # ---- tiny warmup AllToAll: opens the collective channels while
            # compute runs so the real exchange pays less entry latency ----
            warm_in = dram.tile([NCORES * 8, D], BF)
            warm_out = dram.tile([NCORES * 8, D], BF)
            nc.gpsimd.collective_compute(
                "AllToAll", bypass,
                replica_groups=[list(range(NCORES))],
                ins=[warm_in[:, :].opt()],
                outs=[warm_out[:, :].opt()],
            )

            # ---- activations in natural layout (rows on partitions), bf16 ----
            # tx_nat[p, t, d] = tx[t*128+p, d];  a_sel[p, t, j] = a[t*128+p, e0+j]
            tx_nat = singles.tile([128, T, 128], BF)
            a_sel = singles.tile([128, T, EC], FP)    # fp32: ACT scale requirement
            for h in range(2):
                ps = mmps.tile([128, NSPLIT, 512], FP, tag="mm")
                for q in range(4):
                    t = h * 4 + q
                    sl = slice(t * 128, (t + 1) * 128)
                    nc.tensor.matmul(ps[:, q, 0:128 + EC], sb_xT[:, sl], sb_WcatT,
                                     start=True, stop=True)
                for q in range(4):
                    t = h * 4 + q
                    nc.vector.tensor_add(tx_nat[:, t, :], ps[:, q, 0:128], sb_bl2B)
                    nc.vector.tensor_add(a_sel[:, t, :], ps[:, q, 128:128 + EC],
                                         sb_blcB)

            # ---- Z term for own shard: Z[c,d] = sum_a mask[c,a] tx[a,d] ----
            z_ps = mmps.tile([128, NSPLIT, 512], FP, tag="mm")
            for t in range(T):
                nc.tensor.matmul(z_ps[:, 0, 0:CB], sb_mTc[:, t, :], tx_nat[:, t, :],
                                 start=(t == 0), stop=(t == T - 1))
            # base = x_shard + blo * Z   (everything not needing the collective)
            base = singles.tile([CB, D], FP)
            nc.vector.scalar_tensor_tensor(base, z_ps[:, 0, 0:CB], 1.0, sb_bloB,
                                           op0=mult, op1=mult)
            nc.vector.tensor_add(base, base, sb_xrow)

            # ---- U'[a, j, d] = a[a,e0+j] * tx[a,d] * Wlo[d,e0+j]  (bf16) ----
            # two stages: atx = a*tx (tensor_scalar, split DVE/ACT), then one
            # big tensor_tensor per a-tile against the flat Wlo broadcast
            atx = singles.tile([128, T, EC * 128], BF)
            up = singles.tile([128, T, EC * 128], BF)
            N_DVE_SCALE = 11   # of the 16 per-tile a-scales, how many go to DVE
            for t in range(T):
                for j in range(EC):
                    dst = atx[:, t, j * 128:(j + 1) * 128]
                    sca = a_sel[:, t, j:j + 1]
                    if j < N_DVE_SCALE:
                        nc.vector.tensor_scalar_mul(dst, tx_nat[:, t, :], sca)
                    else:
                        nc.scalar.mul(dst, tx_nat[:, t, :], sca)
                nc.vector.tensor_mul(up[:, t, :], atx[:, t, :], sb_WloBc)

            # ---- main loop over output c-tiles. The first two c-tiles are
            # interleaved with the a-tile loop outermost so the matmuls
            # consume U' tiles as the vector engines produce them; the rest
            # run sequentially with PSUM slot rotation hiding the V drain ----
            part_dram = dram.tile([L, D], BF)
            a2a_dram = dram.tile([L, D], BF)

            def v_stage(ct, ps, n_act=6):
                # V[c, j, d] = a[c, e0+j] * T'[c, j, d]  (ACT/DVE, PSUM->SBUF)
                v = vpool.tile([128, EC, 128], BF, tag="v")
                for j in range(EC):
                    src_ap = ps[:, j // 4, (j % 4) * 128:(j % 4 + 1) * 128]
                    sca = a_sel[:, ct, j:j + 1]
                    if j < n_act:
                        nc.scalar.mul(v[:, j, :], src_ap, sca)
                    else:
                        nc.vector.tensor_scalar_mul(v[:, j, :], src_ap, sca)
                # tree-reduce over j -> partial[c, d]
                t8 = trpool.tile([128, 8, 128], BF, tag="t8")
                nc.vector.tensor_add(t8, v[:, 0:8, :], v[:, 8:16, :])
                t4 = trpool.tile([128, 4, 128], BF, tag="t4")
                nc.vector.tensor_add(t4, t8[:, 0:4, :], t8[:, 4:8, :])
                t2 = trpool.tile([128, 2, 128], BF, tag="t2")
                nc.vector.tensor_add(t2, t4[:, 0:2, :], t4[:, 2:4, :])
                p1 = trpool.tile([128, 128], BF, tag="p1")
                nc.vector.tensor_add(p1, t2[:, 0, :], t2[:, 1, :])
                nc.sync.dma_start(part_dram[ct * 128:(ct + 1) * 128, :], p1)

            ps0 = mmps.tile([128, NSPLIT, 512], FP, tag="mm")
            ps1 = mmps.tile([128, NSPLIT, 512], FP, tag="mm")
            for t in range(T):
                for ct, ps in ((0, ps0), (1, ps1)):
                    lhsT = sb_mT[:, t, ct * 128:(ct + 1) * 128]
                    for n in range(NSPLIT):
                        nc.tensor.matmul(
                            ps[:, n, :], lhsT, up[:, t, n * 512:(n + 1) * 512],
                            start=(t == 0), stop=(t == T - 1),
                        )
            v_stage(0, ps0)
            v_stage(1, ps1)

            for ct in range(2, T):
                ps = mmps.tile([128, NSPLIT, 512], FP, tag="mm")
                for t in range(T):
                    lhsT = sb_mT[:, t, ct * 128:(ct + 1) * 128]
                    for n in range(NSPLIT):
                        nc.tensor.matmul(
                            ps[:, n, :], lhsT, up[:, t, n * 512:(n + 1) * 512],
                            start=(t == 0), stop=(t == T - 1),
                        )
                v_stage(ct, ps)

            nc.gpsimd.collective_compute(
                "AllToAll", bypass,
                replica_groups=[list(range(NCORES))],
                ins=[part_dram[:, :].opt()],
                outs=[a2a_dram[:, :].opt()],
            )
            # AllToAll leaves, on core k, the 8 cores' partials of this
            # core's 128-row c-shard: [i, 128, D] with source core i.
            red_sb = singles.tile([128, NCORES, D], BF)
            nc.sync.dma_start(
                red_sb, a2a_dram[:, :].rearrange("(s p) d -> p s d", p=128))
            r4 = singles.tile([128, 4, D], BF)
            nc.vector.tensor_add(r4, red_sb[:, 0:4, :], red_sb[:, 4:8, :])
            r2 = singles.tile([128, 2, D], BF)
            nc.vector.tensor_add(r2, r4[:, 0:2, :], r4[:, 2:4, :])
            y_sb = singles.tile([CB, D], FP)
            nc.vector.tensor_add(y_sb, r2[:, 0, :], r2[:, 1, :])

            # ---- y = base + reduced partials ; LayerNorm ----
            nc.vector.tensor_add(y_sb, y_sb, base)

            stats = singles.tile([CB, nc.vector.BN_STATS_DIM], FP)
            nc.vector.bn_stats(stats, y_sb)
            mv = singles.tile([CB, 2], FP)
            nc.vector.bn_aggr(mv, stats)
            sd = singles.tile([CB, 1], FP)
            nc.scalar.activation(sd, mv[:, 1:2], Sqrt, bias=sb_eps, scale=1.0)
            rstd = singles.tile([CB, 1], FP)
            nc.vector.reciprocal(rstd, sd)
            nc.vector.tensor_scalar(y_sb, y_sb, mv[:, 0:1], rstd,
                                    op0=mybir.AluOpType.subtract, op1=mult)
            nc.vector.tensor_mul(y_sb, y_sb, sb_gam)
            nc.vector.tensor_add(y_sb, y_sb, sb_bet)

            nc.sync.dma_start(out[:, :], y_sb)

    return nc


_NC_CACHE = None


def _get_nc():
    global _NC_CACHE
    if _NC_CACHE is None:
        _NC_CACHE = build_nc()
        _NC_CACHE.finalize()
    return _NC_CACHE


def _prepare_in_maps(x, mask, Wl, bl, Wlo, blo, Wl2, bl2, gamma, beta):
    f32 = np.float32
    bf16 = ml_dtypes.bfloat16
    x0 = np.ascontiguousarray(np.asarray(x, f32)[0])          # [L, D]
    xT = np.ascontiguousarray(x0.T)                           # [128, L]
    m = np.asarray(mask)[0].astype(bf16)                      # [L(c), L(a)]
    # mT[p, t, c] = mask[c, t*128 + p]
    mT_full = np.ascontiguousarray(
        m.T.reshape(T, 128, L).transpose(1, 0, 2))            # [128, T, L]
    WlT = np.ascontiguousarray(np.asarray(Wl, f32).T)         # [in, e]
    Wl2T = np.ascontiguousarray(np.asarray(Wl2, f32).T)
    WloT = np.asarray(Wlo, f32).T                             # [e, d]
    bl_ = np.asarray(bl, f32)
    bl2B = np.ascontiguousarray(np.broadcast_to(np.asarray(bl2, f32), (128, 128)))
    bloB = np.ascontiguousarray(np.broadcast_to(np.asarray(blo, f32), (CB, D)))
    gam_b = np.ascontiguousarray(np.broadcast_to(np.asarray(gamma, f32), (CB, D)))
    bet_b = np.ascontiguousarray(np.broadcast_to(np.asarray(beta, f32), (CB, D)))

    in_maps = []
    for k in range(NCORES):
        own = np.r_[k * CB:(k + 1) * CB]
        e0 = k * EC
        WloBc = np.ascontiguousarray(
            np.broadcast_to(WloT[e0:e0 + EC].astype(bf16).reshape(1, EC * 128),
                            (128, EC * 128)))
        in_maps.append({
            "xT": xT,
            "xrow": np.ascontiguousarray(x0[own]),
            "mT": mT_full,
            "mTc": np.ascontiguousarray(mT_full[:, :, own]),
            "WcatT": np.ascontiguousarray(
                np.concatenate([Wl2T, WlT[:, e0:e0 + EC]], axis=1)),
            "WloBc": WloBc,
            "blcB": np.ascontiguousarray(
                np.broadcast_to(bl_[e0:e0 + EC], (128, EC))),
            "bl2B": bl2B,
            "bloB": bloB,
            "gam": gam_b,
            "bet": bet_b,
        })
    return in_maps


def kernel(x, mask, Wl, bl, Wlo, blo, Wl2, bl2, gamma, beta):
    in_maps = _prepare_in_maps(x, mask, Wl, bl, Wlo, blo, Wl2, bl2, gamma, beta)
    res = run_bass_kernel_spmd(_get_nc(), in_maps, core_ids=list(range(NCORES)))
    y = np.concatenate([res.results[k]["out"] for k in range(NCORES)], axis=0)
    return y.reshape(B, L, D).astype(np.float32)


# revision 27
# speedup vs baseline: 1.2463x; 1.0612x over previous
"""Trainium2 Bass kernel for nn_JResCOPAttn (B=1, L=1024, D=128).

Reference computation:
    a   = x @ Wl.T + bl                                # [L, D]
    tm  = (a[:,None,:] * a[None,:,:]) @ Wlo.T + blo    # [L, L, D]  (never materialized!)
    tm *= (mask != 0)
    tx  = x @ Wl2.T + bl2                              # [L, D]
    y   = x + einsum('cad,ad->cd', tm, tx)
    out = LayerNorm(y) * gamma + beta

Algebraic restructuring (e indexes the D channels of `a`):
    y1[c,d] = sum_e a[c,e] * T'[c,e,d]  +  blo[d] * Z[c,d]
    T'[c,e,d] = sum_a mask[c,a] * (a[a,e] * tx[a,d] * Wlo[d,e])
    Z[c,d]    = sum_a mask[c,a] * tx[a,d]

Sharding: the e axis (128 channels) is split across the 8 cores (16 each).
Every core computes its 16-channel partial y1 for ALL 1024 output rows:
    U'[a, j, d] = a[a, e0+j] * tx[a,d] * Wlo[d, e0+j]   (bf16, DVE fused op)
    T' = mask @ U'                                      (bf16 matmuls, N=512)
    partial[c,d] = sum_j a[c, e0+j] * T'[c,j,d]         (ACT scale + DVE tree)
then an AllToAll over HBM hands each core the 8 per-core partials of its own
128-row c-shard, which it sums locally before the Z term, residual and
LayerNorm are applied.
The per-core e-chunk enters only through input tensors (WlTc / WloBc), so a
single compiled program serves all 8 cores.
"""

import os
import sys

for _p in ("/opt/trn_rl_repo", "/root/.axon_site/_ro/trn_rl_repo"):
    if os.path.isdir(_p) and _p not in sys.path:
        sys.path.insert(0, _p)

import numpy as np
import ml_dtypes

import concourse.tile as tile
from concourse import bacc, mybir
from concourse.bass_utils import run_bass_kernel_spmd

B, L, D = 1, 1024, 128
NCORES = 8
CB = L // NCORES          # c-rows per core shard = 128
T = L // 128              # a-tiles / c-tiles = 8
EC = D // NCORES          # e-channels per core = 16
EPS = 1e-5
FP = mybir.dt.float32
BF = mybir.dt.bfloat16

NSPLIT = 4                # 512-wide n-chunks per (ct, t) matmul group


def build_nc():
    nc = bacc.Bacc("TRN2", target_bir_lowering=False, num_devices=NCORES,
                   num_swdge_queues=4)

    # ---- I/O (per-core tensors; e-chunk/c-shard baked into the data) ----
    xT    = nc.dram_tensor("xT",    [128, L], FP, kind="ExternalInput")      # x^T
    xrow  = nc.dram_tensor("xrow",  [CB, D], FP, kind="ExternalInput")       # own c-shard of x
    mT    = nc.dram_tensor("mT",    [128, T, L], BF, kind="ExternalInput")   # mT[p,t,c] = mask[c, t*128+p]
    mTc   = nc.dram_tensor("mTc",   [128, T, CB], BF, kind="ExternalInput")  # own-shard mask cols
    WcatT = nc.dram_tensor("WcatT", [128, 128 + EC], FP, kind="ExternalInput")  # [Wl2.T | Wl.T[:,e0:e0+EC]]
    WloBc = nc.dram_tensor("WloBc", [128, EC * 128], BF, kind="ExternalInput")  # Wlo[d, e0+j], bcast parts
    blcB  = nc.dram_tensor("blcB",  [128, EC], FP, kind="ExternalInput")     # bl[e0:e0+EC] bcast parts
    bl2B  = nc.dram_tensor("bl2B",  [128, 128], FP, kind="ExternalInput")    # bl2 bcast parts
    bloB  = nc.dram_tensor("bloB",  [CB, D], FP, kind="ExternalInput")       # blo bcast parts
    gam   = nc.dram_tensor("gam",   [CB, D], FP, kind="ExternalInput")
    bet   = nc.dram_tensor("bet",   [CB, D], FP, kind="ExternalInput")
    out   = nc.dram_tensor("out",   [CB, D], FP, kind="ExternalOutput")

    Sqrt = mybir.ActivationFunctionType.Sqrt
    mult = mybir.AluOpType.mult
    add = mybir.AluOpType.add
    bypass = mybir.AluOpType.bypass

    with tile.TileContext(nc) as tc:
        with (
            tc.tile_pool(name="singles", bufs=1) as singles,
            tc.tile_pool(name="mm", bufs=2, space="PSUM") as mmps,
            tc.tile_pool(name="v", bufs=2) as vpool,
            tc.tile_pool(name="tr", bufs=2) as trpool,
            tc.tile_pool(name="dram", bufs=1, space="DRAM") as dram,
        ):
            # ---- load inputs ----
            sb_xT = singles.tile([128, L], FP)
            nc.sync.dma_start(sb_xT, xT[:, :])
            sb_WcatT = singles.tile([128, 128 + EC], FP)
            nc.sync.dma_start(sb_WcatT, WcatT[:, :])
            sb_WloBc = singles.tile([128, EC * 128], BF)
            nc.sync.dma_start(sb_WloBc, WloBc[:, :])
            sb_blcB = singles.tile([128, EC], FP)
            nc.sync.dma_start(sb_blcB, blcB[:, :])
            sb_bl2B = singles.tile([128, 128], FP)
            nc.sync.dma_start(sb_bl2B, bl2B[:, :])
            sb_bloB = singles.tile([CB, D], FP)
            nc.sync.dma_start(sb_bloB, bloB[:, :])
            sb_xrow = singles.tile([CB, D], FP)
            nc.sync.dma_start(sb_xrow, xrow[:, :])
            sb_gam = singles.tile([CB, D], FP)
            nc.sync.dma_start(sb_gam, gam[:, :])
            sb_bet = singles.tile([CB, D], FP)
            nc.sync.dma_start(sb_bet, bet[:, :])
            sb_eps = singles.tile([CB, 1], FP)
            nc.vector.memset(sb_eps, EPS)
            sb_mT = singles.tile([128, T, L], BF)
            nc.sync.dma_start(sb_mT, mT[:, :, :])
            sb_mTc = singles.tile([128, T, CB], BF)
            nc.sync.dma_start(sb_mTc, mTc[:, :, :])

            # ---- tiny warmup AllToAll: opens the collective channels while
            # compute runs so the real exchange pays less entry latency ----
            warm_in = dram.tile([NCORES * 8, D], BF)
            warm_out = dram.tile([NCORES * 8, D], BF)
            nc.gpsimd.collective_compute(
                "AllToAll", bypass,
                replica_groups=[list(range(NCORES))],
                ins=[warm_in[:, :].opt()],
                outs=[warm_out[:, :].opt()],
            )

            # ---- activations in natural layout (rows on partitions), bf16 ----
            # tx_nat[p, t, d] = tx[t*128+p, d];  a_sel[p, t, j] = a[t*128+p, e0+j]
            tx_nat = singles.tile([128, T, 128], BF)
            a_sel = singles.tile([128, T, EC], FP)    # fp32: ACT scale requirement
            for h in range(2):
                ps = mmps.tile([128, NSPLIT, 512], FP, tag="mm")
                for q in range(4):
                    t = h * 4 + q
                    sl = slice(t * 128, (t + 1) * 128)
                    nc.tensor.matmul(ps[:, q, 0:128 + EC], sb_xT[:, sl], sb_WcatT,
                                     start=True, stop=True)
                for q in range(4):
                    t = h * 4 + q
                    nc.vector.tensor_add(tx_nat[:, t, :], ps[:, q, 0:128], sb_bl2B)
                    nc.vector.tensor_add(a_sel[:, t, :], ps[:, q, 128:128 + EC],
                                         sb_blcB)

            # ---- Z term for own shard: Z[c,d] = sum_a mask[c,a] tx[a,d] ----
            z_ps = mmps.tile([128, NSPLIT, 512], FP, tag="mm")
            for t in range(T):
                nc.tensor.matmul(z_ps[:, 0, 0:CB], sb_mTc[:, t, :], tx_nat[:, t, :],
                                 start=(t == 0), stop=(t == T - 1))
            # base = x_shard + blo * Z   (everything not needing the collective)
            base = singles.tile([CB, D], FP)
            nc.vector.scalar_tensor_tensor(base, z_ps[:, 0, 0:CB], 1.0, sb_bloB,
                                           op0=mult, op1=mult)
            nc.vector.tensor_add(base, base, sb_xrow)

            # ---- U'[a, j, d] = a[a,e0+j] * tx[a,d] * Wlo[d,e0+j]  (bf16) ----
            # two stages: atx = a*tx (tensor_scalar, split DVE/ACT), then one
            # big tensor_tensor per a-tile against the flat Wlo broadcast
            atx = singles.tile([128, T, EC * 128], BF)
            up = singles.tile([128, T, EC * 128], BF)
            N_DVE_SCALE = 11   # of the 16 per-tile a-scales, how many go to DVE
            for t in range(T):
                for n in range(NSPLIT):
                    for jq in range(4):
                        j = n * 4 + jq
                        dst = atx[:, t, j * 128:(j + 1) * 128]
                        sca = a_sel[:, t, j:j + 1]
                        if j < N_DVE_SCALE:
                            nc.vector.tensor_scalar_mul(dst, tx_nat[:, t, :], sca)
                        else:
                            nc.scalar.mul(dst, tx_nat[:, t, :], sca)
                    sl = slice(n * 512, (n + 1) * 512)
                    nc.vector.tensor_mul(up[:, t, sl], atx[:, t, sl],
                                         sb_WloBc[:, sl])

            # ---- main loop over output c-tiles. The first two c-tiles are
            # interleaved with the a-tile loop outermost so the matmuls
            # consume U' tiles as the vector engines produce them; the rest
            # run sequentially with PSUM slot rotation hiding the V drain ----
            part_dram = dram.tile([L, D], BF)
            a2a_dram = dram.tile([L, D], BF)

            def v_stage(ct, ps, n_act=6):
                # V[c, j, d] = a[c, e0+j] * T'[c, j, d]  (ACT/DVE, PSUM->SBUF)
                v = vpool.tile([128, EC, 128], BF, tag="v")
                for j in range(EC):
                    src_ap = ps[:, j // 4, (j % 4) * 128:(j % 4 + 1) * 128]
                    sca = a_sel[:, ct, j:j + 1]
                    if j < n_act:
                        nc.scalar.mul(v[:, j, :], src_ap, sca)
                    else:
                        nc.vector.tensor_scalar_mul(v[:, j, :], src_ap, sca)
                # tree-reduce over j -> partial[c, d]
                t8 = trpool.tile([128, 8, 128], BF, tag="t8")
                nc.vector.tensor_add(t8, v[:, 0:8, :], v[:, 8:16, :])
                t4 = trpool.tile([128, 4, 128], BF, tag="t4")
                nc.vector.tensor_add(t4, t8[:, 0:4, :], t8[:, 4:8, :])
                t2 = trpool.tile([128, 2, 128], BF, tag="t2")
                nc.vector.tensor_add(t2, t4[:, 0:2, :], t4[:, 2:4, :])
                p1 = trpool.tile([128, 128], BF, tag="p1")
                nc.vector.tensor_add(p1, t2[:, 0, :], t2[:, 1, :])
                nc.sync.dma_start(part_dram[ct * 128:(ct + 1) * 128, :], p1)

            ps0 = mmps.tile([128, NSPLIT, 512], FP, tag="mm")
            ps1 = mmps.tile([128, NSPLIT, 512], FP, tag="mm")
            for t in range(T):
                for ct, ps in ((0, ps0), (1, ps1)):
                    lhsT = sb_mT[:, t, ct * 128:(ct + 1) * 128]
                    for n in range(NSPLIT):
                        nc.tensor.matmul(
                            ps[:, n, :], lhsT, up[:, t, n * 512:(n + 1) * 512],
                            start=(t == 0), stop=(t == T - 1),
                        )
            v_stage(0, ps0)
            v_stage(1, ps1)

            for ct in range(2, T):
                ps = mmps.tile([128, NSPLIT, 512], FP, tag="mm")
                for t in range(T):
                    lhsT = sb_mT[:, t, ct * 128:(ct + 1) * 128]
                    for n in range(NSPLIT):
                        nc.tensor.matmul(
                            ps[:, n, :], lhsT, up[:, t, n * 512:(n + 1) * 512],
                            start=(t == 0), stop=(t == T - 1),
                        )
                v_stage(ct, ps)

            nc.gpsimd.collective_compute(
                "AllToAll", bypass,
                replica_groups=[list(range(NCORES))],
                ins=[part_dram[:, :].opt()],
                outs=[a2a_dram[:, :].opt()],
            )
            # AllToAll leaves, on core k, the 8 cores' partials of this
            # core's 128-row c-shard: [i, 128, D] with source core i.
            red_sb = singles.tile([128, NCORES, D], BF)
            nc.sync.dma_start(
                red_sb, a2a_dram[:, :].rearrange("(s p) d -> p s d", p=128))
            r4 = singles.tile([128, 4, D], BF)
            nc.vector.tensor_add(r4, red_sb[:, 0:4, :], red_sb[:, 4:8, :])
            r2 = singles.tile([128, 2, D], BF)
            nc.vector.tensor_add(r2, r4[:, 0:2, :], r4[:, 2:4, :])
            y_sb = singles.tile([CB, D], FP)
            nc.vector.tensor_add(y_sb, r2[:, 0, :], r2[:, 1, :])

            # ---- y = base + reduced partials ; LayerNorm ----
            nc.vector.tensor_add(y_sb, y_sb, base)

            stats = singles.tile([CB, nc.vector.BN_STATS_DIM], FP)
            nc.vector.bn_stats(stats, y_sb)
            mv = singles.tile([CB, 2], FP)
            nc.vector.bn_aggr(mv, stats)
            sd = singles.tile([CB, 1], FP)
            nc.scalar.activation(sd, mv[:, 1:2], Sqrt, bias=sb_eps, scale=1.0)
            rstd = singles.tile([CB, 1], FP)
            nc.vector.reciprocal(rstd, sd)
            nc.vector.tensor_scalar(y_sb, y_sb, mv[:, 0:1], rstd,
                                    op0=mybir.AluOpType.subtract, op1=mult)
            nc.vector.tensor_mul(y_sb, y_sb, sb_gam)
            nc.vector.tensor_add(y_sb, y_sb, sb_bet)

            nc.sync.dma_start(out[:, :], y_sb)

    return nc


_NC_CACHE = None


def _get_nc():
    global _NC_CACHE
    if _NC_CACHE is None:
        _NC_CACHE = build_nc()
        _NC_CACHE.finalize()
    return _NC_CACHE


def _prepare_in_maps(x, mask, Wl, bl, Wlo, blo, Wl2, bl2, gamma, beta):
    f32 = np.float32
    bf16 = ml_dtypes.bfloat16
    x0 = np.ascontiguousarray(np.asarray(x, f32)[0])          # [L, D]
    xT = np.ascontiguousarray(x0.T)                           # [128, L]
    m = np.asarray(mask)[0].astype(bf16)                      # [L(c), L(a)]
    # mT[p, t, c] = mask[c, t*128 + p]
    mT_full = np.ascontiguousarray(
        m.T.reshape(T, 128, L).transpose(1, 0, 2))            # [128, T, L]
    WlT = np.ascontiguousarray(np.asarray(Wl, f32).T)         # [in, e]
    Wl2T = np.ascontiguousarray(np.asarray(Wl2, f32).T)
    WloT = np.asarray(Wlo, f32).T                             # [e, d]
    bl_ = np.asarray(bl, f32)
    bl2B = np.ascontiguousarray(np.broadcast_to(np.asarray(bl2, f32), (128, 128)))
    bloB = np.ascontiguousarray(np.broadcast_to(np.asarray(blo, f32), (CB, D)))
    gam_b = np.ascontiguousarray(np.broadcast_to(np.asarray(gamma, f32), (CB, D)))
    bet_b = np.ascontiguousarray(np.broadcast_to(np.asarray(beta, f32), (CB, D)))

    in_maps = []
    for k in range(NCORES):
        own = np.r_[k * CB:(k + 1) * CB]
        e0 = k * EC
        WloBc = np.ascontiguousarray(
            np.broadcast_to(WloT[e0:e0 + EC].astype(bf16).reshape(1, EC * 128),
                            (128, EC * 128)))
        in_maps.append({
            "xT": xT,
            "xrow": np.ascontiguousarray(x0[own]),
            "mT": mT_full,
            "mTc": np.ascontiguousarray(mT_full[:, :, own]),
            "WcatT": np.ascontiguousarray(
                np.concatenate([Wl2T, WlT[:, e0:e0 + EC]], axis=1)),
            "WloBc": WloBc,
            "blcB": np.ascontiguousarray(
                np.broadcast_to(bl_[e0:e0 + EC], (128, EC))),
            "bl2B": bl2B,
            "bloB": bloB,
            "gam": gam_b,
            "bet": bet_b,
        })
    return in_maps


def kernel(x, mask, Wl, bl, Wlo, blo, Wl2, bl2, gamma, beta):
    in_maps = _prepare_in_maps(x, mask, Wl, bl, Wlo, blo, Wl2, bl2, gamma, beta)
    res = run_bass_kernel_spmd(_get_nc(), in_maps, core_ids=list(range(NCORES)))
    y = np.concatenate([res.results[k]["out"] for k in range(NCORES)], axis=0)
    return y.reshape(B, L, D).astype(np.float32)
